# revision 54
# baseline (speedup 1.0000x reference)
"""Trainium2 Bass kernel for the segment-reduce cosine loss problem.

Reference computation (per sample b, S=32 labels):
  onehot[l,s] = (attributes[b,l] == s+1)
  seg_sum[s,:] = sum_l onehot[l,s] * text_feats[b,l,:]
  cos[s] = <Vgs[b,s], seg_sum[s]> / (|Vgs[b,s]| * |seg_sum[s]|)
  loss = 1 - mean cos  (cosine is scale-invariant in seg_mean, so seg_sum
  substitutes for seg_mean; the reference's 1e-8 clamp is unreachable)

Sharding: pure data parallel over batch; each of 8 cores processes 8
samples. The on-device kernel performs the entire O(B*L*D) segment
reduction and ships each sample's seg_sum (transposed, bf16) back; the
host finishes the O(B*S*D) cosine/loss assembly in numpy.

Per-core kernel design (Tile framework on bacc):
  - text_feats stream in through gpsimd (SWDGE) casting DMAs f32->fp8e3
    (e3m4: ~1.4e-2 elementwise rounding, ~1.2e-5 on the final loss).
    Samples 0..6 are monolithic per-sample DMAs (sample 0 in two halves
    so the first descriptor-gen is shorter); sample 7 streams as chunk
    groups (c0-3, c4-6) with the final chunk split into two 512-column
    halves (512B descriptor elements - no sub-512B penalty) so only the
    last half's four matmuls trail the stream.
  - matmuls run text-stationary: lhsT = text block [128 tok, 128 feat]
    (ldweights), rhs = onehot [128 tok, 32] (moving) -> PSUM accumulates
    seg_sum^T blocks [128 feat, 32 lbl]. PE cost scales with S=32 moving
    rows per block instead of 512, keeping the PE off the critical path
    at fp8 stream rates.
  - PSUM accumulation groups are per-bank (a start matmul zeroes the
    whole 2KB bank region), so each sample uses a [128, 2048] 4-bank
    PSUM tile with TWO seg_sum^T blocks per bank (cols j*512+{0:32,32:64})
    under a single start/stop bracket per bank. 4-bank footprint = two
    samples accumulate concurrently (bufs=2, ring shared with the
    attribute-transpose PSUM), and each sample's copy-out is ONE strided
    op into a bf16 staging tile (split copies get re-serialized by the
    framework's cross-engine clock alignment).
  - outs are HWDGE DMAs that overlap the stream, except sample 3's which
    goes through the Pool queue after all text desc-gens: that puts the 8
    HWDGE DMAs at attr + 7 outs, so sample 7's out owns HWDGE lane 7,
    whose completion the end-of-kernel drain chain checks LAST - the
    other lane checks clear while it waits instead of running after.
  - scheduling constraints found by trace iteration: SWDGE desc-gens
    round-robin over 8 DMASW lanes and a lane's next DMA waits its
    ancestor's completion (+900ns sem), so the Pool DMA count stays at 15
    with the tail pieces' ancestors completing early; tile-pool bufs are
    sized so every desc-gen runs ahead without buffer waits. Merging
    small pieces (fewer Pool DMAs) repeatedly beat finer splits.
"""

import numpy as np

import concourse.mybir as mybir
import concourse.tile as tile
from concourse import bacc
from concourse.bass_utils import run_bass_kernel_spmd

B, L, D, S = 64, 1024, 1024, 32
N_CORES = 8
BPC = B // N_CORES        # samples per core
NCHUNK = L // 128         # token chunks of 128 positions
NBLK = D // 128           # feature blocks of 128 columns
NBANK = NBLK // 2         # psum banks per sample (2 blocks per bank)
BANKC = 512               # f32 columns per psum bank
EPS = 1e-8

F32 = mybir.dt.float32
I32 = mybir.dt.int32
DT_LO = mybir.dt.float8e3    # text stream dtype (e3m4)
DT_SHIP = mybir.dt.bfloat16  # seg_sum^T shipping dtype
ALU = mybir.AluOpType

# sample 7's final chunk is split at this block boundary into two
# pieces (blocks [0, TAIL_BLK) and [TAIL_BLK, NBLK)); 4 gives two
# 512-column halves whose 512B descriptor elements avoid the sub-512B
# DMA penalty while only four matmuls trail the final piece
TAIL_BLK = 4


def build_bass():
    nc = bacc.Bacc(
        "TRN2", target_bir_lowering=False, debug=False, num_devices=N_CORES
    )
    attrs_d = nc.dram_tensor("attributes", [BPC, L], I32, kind="ExternalInput")
    text_d = nc.dram_tensor("text_feats", [BPC, L, D], F32, kind="ExternalInput")
    out_d = nc.dram_tensor("out", [BPC, 128, NBLK * S], DT_SHIP, kind="ExternalOutput")

    with tile.TileContext(nc) as tc:
        with (
            tc.tile_pool(name="const", bufs=1) as const_pool,
            tc.tile_pool(name="s0", bufs=2) as s0_pool,
            tc.tile_pool(name="text", bufs=6) as text_pool,
            tc.tile_pool(name="s7", bufs=NCHUNK + 2) as s7_pool,
            tc.tile_pool(name="oh", bufs=8) as oh_pool,
            tc.tile_pool(name="stage", bufs=8) as stage_pool,
            tc.tile_pool(name="psum", bufs=2, space="PSUM") as psum_pool,
        ):
            def text_dma(t, b, c_lo, c_hi, d_lo, d_hi):
                # casting DMA (f32 -> DT_LO) of chunk rows c_lo:c_hi,
                # feature cols d_lo:d_hi into tile t [128, nc*ncol].
                # The destination AP stays FLAT: its innermost contiguous
                # run is then the whole per-partition span, which raises the
                # modeled descriptor element size (fewer, larger SWDGE
                # descriptors -> shorter desc-gen on the Pool sequencer).
                # Element order matches the [p][c][d] source iteration.
                nc.gpsimd.dma_start(
                    t[:],
                    text_d[b, c_lo * 128:c_hi * 128, d_lo:d_hi]
                    .rearrange("(c p) d -> p c d", p=128),
                )

            # sample 0 halves first: the first SWDGE desc-gen gates the head
            tx0 = []
            for h in range(2):
                t = s0_pool.tile([128, (NCHUNK // 2) * D], DT_LO, tag="s0tx", name=f"tx0_{h}")
                text_dma(t, 0, h * (NCHUNK // 2), (h + 1) * (NCHUNK // 2), 0, D)
                tx0.append(t)

            # ---- constants (on Pool after the first text gens) ----
            # two iotas only: ident's row-index values come from iota_s
            # (both count 1..N along the free dim)
            iota_s = const_pool.tile([128, S], F32, name="iota_s")
            nc.gpsimd.iota(
                iota_s[:], pattern=[[1, S]], base=1, channel_multiplier=0,
                allow_small_or_imprecise_dtypes=True,
            )
            idcol = const_pool.tile([BPC, 1], F32, name="idcol")
            nc.gpsimd.iota(
                idcol[:], pattern=[[0, 1]], base=1, channel_multiplier=1,
                allow_small_or_imprecise_dtypes=True,
            )
            ident = const_pool.tile([BPC, BPC], F32, name="ident")
            nc.vector.tensor_tensor(
                ident[:], idcol[:, 0:1].broadcast_to([BPC, BPC]),
                iota_s[0:BPC, 0:BPC], op=ALU.is_equal,
            )
            # ---- attribute prep: transpose so token position lands on a
            # partition: attr_sb[p, b*NCHUNK + c] = attributes[b, c*128 + p]
            attr_i = const_pool.tile([BPC, L], I32, name="attr_i")
            nc.scalar.dma_start(attr_i[:], attrs_d[:])
            attr_f = const_pool.tile([BPC, L], F32, name="attr_f")
            nc.vector.tensor_copy(attr_f[:], attr_i[:])
            psum_attr = psum_pool.tile([128, NCHUNK * BPC], F32, tag="ps", name="psum_attr")
            for c in range(NCHUNK):
                nc.tensor.transpose(
                    psum_attr[:, c * BPC:(c + 1) * BPC],
                    attr_f[:, c * 128:(c + 1) * 128],
                    ident[:],
                )
            attr_sb = const_pool.tile([128, BPC * NCHUNK], F32, name="attr_sb")
            nc.vector.tensor_copy(
                attr_sb[:].rearrange("p (b c) -> p c b", c=NCHUNK),
                psum_attr[:].rearrange("p (c b) -> p c b", b=BPC),
            )

            # ---- per-sample onehot blocks: oh[p, c, s] = (attr == s+1)
            oh_tiles = []
            for b in range(BPC):
                oh_all = oh_pool.tile([128, NCHUNK * S], DT_LO, tag="oh", name=f"oh_{b}")
                nc.vector.tensor_tensor(
                    oh_all[:].rearrange("p (c s) -> p c s", s=S),
                    attr_sb[:, b * NCHUNK:(b + 1) * NCHUNK]
                    .unsqueeze(2).broadcast_to([128, NCHUNK, S]),
                    iota_s[:].unsqueeze(1).broadcast_to([128, NCHUNK, S]),
                    op=ALU.is_equal,
                )
                oh_tiles.append(oh_all)

            # psum col offset of block k: bank k//2, sub-slot k%2
            def blk_cols(k):
                lo = (k // 2) * BANKC + (k % 2) * S
                return slice(lo, lo + S)

            def mm(ps, blk, lhsT, ohr, start, stop):
                nc.tensor.matmul(ps[:, blk_cols(blk)], lhsT, ohr, start=start, stop=stop)

            def copy_and_stage(b, ps, eng=None):
                # one strided copy psum -> bf16 staging (alternate DVE/ACT
                # across samples); a single op avoids WAR serialization
                # between sub-tile staging writes
                st = stage_pool.tile([128, NBLK * S], DT_SHIP, tag="st", name=f"st_{b}")
                if eng is None:
                    eng = nc.vector.tensor_copy if (b % 2 == 0) else nc.scalar.copy
                eng(
                    st[:].rearrange("p (k s) -> p k s", s=2 * S),
                    ps[:].rearrange("p (k c) -> p k c", c=BANKC)[:, :, 0:2 * S],
                )
                return st

            # ---- samples 0..6: monolithic stream ----
            stages = {}
            for b in range(7):
                if b == 0:
                    tx_sl = lambda c, blk: tx0[c // (NCHUNK // 2)][
                        :, (c % (NCHUNK // 2)) * D + blk * 128:
                        (c % (NCHUNK // 2)) * D + (blk + 1) * 128]
                else:
                    tx = text_pool.tile([128, NCHUNK * D], DT_LO, tag="tx", name=f"tx_{b}")
                    text_dma(tx, b, 0, NCHUNK, 0, D)
                    tx_sl = lambda c, blk, tx=tx: tx[:, c * D + blk * 128:c * D + (blk + 1) * 128]
                ps = psum_pool.tile([128, NBANK * BANKC], F32, tag="ps", name=f"ps_{b}")
                for c in range(NCHUNK):
                    ohr = oh_tiles[b][:, c * S:(c + 1) * S]
                    for blk in range(NBLK):
                        mm(ps, blk, tx_sl(c, blk), ohr,
                           start=(c == 0 and blk % 2 == 0),
                           stop=(c == NCHUNK - 1 and blk % 2 == 1))
                stages[b] = copy_and_stage(b, ps)

            # ---- sample 7: per-chunk stream; final chunk split by feature
            # columns so the last-arriving piece is one block ----
            b = 7
            ps7 = psum_pool.tile([128, NBANK * BANKC], F32, tag="ps", name="ps_7")
            # chunks 0-3 in one DMA (lands early), 4-6 per chunk: keeps the
            # Pool DMA count low enough that SWDGE lane ancestors (8-lane
            # round-robin, reuse gated on ancestor completion) finish early
            t03 = s7_pool.tile([128, 4 * D], DT_LO, tag="s7tx", name="s7tx_c03")
            text_dma(t03, b, 0, 4, 0, D)
            t46 = s7_pool.tile([128, 3 * D], DT_LO, tag="s7tx", name="s7tx_c46")
            text_dma(t46, b, 4, NCHUNK - 1, 0, D)
            chunk_tiles = [t03[:, c * D:(c + 1) * D] for c in range(4)] + [
                t46[:, c * D:(c + 1) * D] for c in range(3)
            ]
            c = NCHUNK - 1
            t_head = s7_pool.tile([128, TAIL_BLK * 128], DT_LO, tag="s7tx", name="s7tx_head")
            text_dma(t_head, b, c, c + 1, 0, TAIL_BLK * 128)
            t_tail_w = (NBLK - TAIL_BLK) * 128
            tt = s7_pool.tile([128, t_tail_w], DT_LO, tag="s7tx", name="s7tx_tail")
            text_dma(tt, b, c, c + 1, TAIL_BLK * 128, D)
            t_tail = [
                tt[:, (k - TAIL_BLK) * 128:(k - TAIL_BLK + 1) * 128]
                for k in range(TAIL_BLK, NBLK)
            ]

            for c in range(NCHUNK - 1):
                ohr = oh_tiles[b][:, c * S:(c + 1) * S]
                for blk in range(NBLK):
                    mm(ps7, blk, chunk_tiles[c][:, blk * 128:(blk + 1) * 128], ohr,
                       start=(c == 0 and blk % 2 == 0), stop=False)
            c = NCHUNK - 1
            ohr = oh_tiles[b][:, c * S:(c + 1) * S]
            for blk in range(TAIL_BLK):
                mm(ps7, blk, t_head[:, blk * 128:(blk + 1) * 128], ohr,
                   start=False, stop=(blk % 2 == 1))
            for k in range(TAIL_BLK, NBLK):
                mm(ps7, k, t_tail[k - TAIL_BLK], ohr,
                   start=False, stop=(k % 2 == 1))
            # single copy + single out for s7 (split copies get serialized by
            # the framework's clock alignment / staging-tile tracking; two
            # outs serialize their desc-gens on the HWDGE device)
            stages[7] = copy_and_stage(7, ps7, eng=nc.vector.tensor_copy)
            # ALL outs are issued here, after every text desc-gen, so their
            # transfers queue BEHIND the remaining text on the exclusive DMA
            # device and fire in the tail gap instead of pushing the last
            # text piece 182ns later each. s3's goes through the Pool queue:
            # with it off HWDGE, the 8 HWDGE DMAs are attr + 7 outs and the
            # s7 out lands on HWDGE lane 7 - the lane the end-of-kernel
            # drain chain checks LAST, so its late completion hides the
            # other lane checks instead of preceding them.
            nc.gpsimd.dma_start(out_d[3], stages[3][:])
            for b in (0, 1, 2, 4, 5, 6):
                nc.scalar.dma_start(out_d[b], stages[b][:])
            nc.sync.dma_start(out_d[7], stages[7][:])



    nc.compile()
    return nc


_NC_CACHE = None


def _get_nc():
    global _NC_CACHE
    if _NC_CACHE is None:
        _NC_CACHE = build_bass()
    return _NC_CACHE


def _finish_on_host(seg_outs: list[np.ndarray], Vgs: np.ndarray) -> np.ndarray:
    """seg_outs: per-core [BPC, 128, NBLK*S] seg_sum^T (any float dtype).
    Host computes cos per (sample, label) and the final mean loss."""
    cos_sum = 0.0
    vg = Vgs.astype(np.float64)
    nv = np.linalg.norm(vg, axis=-1)  # [B, S]
    for i, o in enumerate(seg_outs):
        # o[j, p, blk*S + s] = seg_sum[b, s, blk*128 + p]
        o = np.asarray(o, dtype=np.float64).reshape(BPC, 128, NBLK, S)
        seg = o.transpose(0, 3, 2, 1).reshape(BPC, S, D)  # [j, s, d]
        v = vg[i * BPC:(i + 1) * BPC]
        num = (v * seg).sum(-1)
        den = np.maximum(nv[i * BPC:(i + 1) * BPC] * np.linalg.norm(seg, axis=-1), EPS)
        cos_sum += float((num / den).sum())
    return np.asarray(1.0 - cos_sum / (B * S), dtype=np.float32)


def kernel(attributes: np.ndarray, text_feats: np.ndarray, Vgs: np.ndarray) -> np.ndarray:
    assert attributes.shape == (B, L) and attributes.dtype == np.int32
    assert text_feats.shape == (B, L, D)
    assert Vgs.shape == (B, S, D)
    nc = _get_nc()
    in_maps = [
        {
            "attributes": np.ascontiguousarray(attributes[i * BPC:(i + 1) * BPC]),
            "text_feats": np.ascontiguousarray(text_feats[i * BPC:(i + 1) * BPC], dtype=np.float32),
        }
        for i in range(N_CORES)
    ]
    res = run_bass_kernel_spmd(nc, in_maps, core_ids=list(range(N_CORES)))
    return _finish_on_host([r["out"] for r in res.results], np.asarray(Vgs))
